# revision 12
# baseline (speedup 1.0000x reference)
"""CAAN kernel for Trainium2, 8-core data-parallel (one batch row per core).

Math: the reference is
    Q = R Wq^T + bq ; K = R Wk^T + bk ; V = R Wv^T + bv
    E = exp(Q K^T / sqrt(512)) ; saat = E / rowsum(E)
    winner = (saat V) W1^T W2^T + (W2 b1 + b2)

Algebraic collapses (host, fp64):
1. The W1/W2 head is linear, so with c = W1^T W2[0]:
       winner[n] = (sum_m E[n,m] u[m]) / (sum_m E[n,m]) + const,
   u = V c = R (Wv^T c) + bv.c — a per-asset scalar.
2. gamma = Q K^T = R A R^T + t[n] + v[m] + bq.bk with A = Wq^T Wk.
   The per-n term t cancels in the s/rowsum ratio; when bq == 0 (true
   for this reference) v and bq.bk vanish, leaving gamma~ = R A R^T.

Device ("E layout": query index n on partitions, key index m free):
  warm-up: dummy matmuls during the input DMA so the PE HAM clock-gate
           reaches 8/8 before real work; a dummy exp preloads the ACT
           table set.
  phase A: ct[q, n] = (R A*32)[n, q] via fp8e4 DoubleRow matmuls
           (contraction 256/MM), per-512-slice PSUM->fp8 casts
           alternating DVE/Pool so the 2-slot PSUM rotation never
           stalls on a cast.
  phase B: per 128-query chunk: one 4-bank [128, 2048] PSUM tile of
           gamma~ (8 DR matmuls), ONE Exp activation over all 2048
           columns with accum_out = rowsum for free, then the weighted
           row-sum s[n] via scalar_tensor_tensor (mult + accum) split
           half on DVE, half on Pool.
  out: s halves and rowsum columns [128, 16] f32; host adds halves and
       does winner = s/rowsum + const.

fp8: A pre-scaled by 32 clears the e4m3 denormal floor (entries ~0.016);
the inverse rides the exp scale. Measured rel err ~3e-3 (tol 2e-2).
"""

import math

import ml_dtypes
import numpy as np

import concourse.ap_utils as ap_utils
import concourse.bass as bass
import concourse.mybir as mybir
import concourse.tile as tile
from concourse.bass_utils import run_bass_kernel_spmd
from concourse.vector_clock import ScopedClock


N_CORES = 8
NB, NN, DD = 8, 2048, 512  # batch, assets, feature dim
P = 128
NQ = DD // P   # q chunks (contraction)
NC = NN // P   # n chunks (query rows)
S = 512        # matmul moving free dim / PSUM bank width
NS = NN // S   # slices of 512 along the free axis
HALF = NN // 2
BF16 = mybir.dt.bfloat16
FP8 = mybir.dt.float8e4
F32 = mybir.dt.float32
SCALE = 1.0 / math.sqrt(float(DD))
ASCALE = 32.0
N_WARM = 14    # dummy matmuls to lift the HAM clock gate before phase A
QS = 1536      # DVE handles et[:, :QS]; gpsimd reduces the rest
BF = ml_dtypes.bfloat16
F8 = ml_dtypes.float8_e4m3
DR = mybir.MatmulPerfMode.DoubleRow


class _TileContext(tile.TileContext):
    """Workaround for walrus rejecting >1 sem wait on the kernel-tail Drain
    ("Too many sync wait commands"): put each final wait on its own SP NoOp
    ahead of an unwaited Drain."""

    def _drain_and_barrier(self, tick_clock, wait_clock):
        nc = self.nc
        probe = nc.sync.nop(nofuse=True)
        wait_clock.add_sem_waits(
            probe.ins, ScopedClock({None: tick_clock.global_clock})
        )
        si = probe.ins.sync_info
        waits = list(si.on_wait) if si is not None else []
        if si is not None:
            si.on_wait = []
        # spread the final waits round-robin over all engines so they
        # resolve in parallel; the barrier then guarantees every wait has
        # been observed before the SP drain runs.
        engines = [nc.sync, nc.vector, nc.scalar, nc.tensor, nc.gpsimd]
        for i, w in enumerate(waits):
            n = engines[i % len(engines)].nop(nofuse=True)
            n.ins.sync_info = mybir.SyncInfo(on_wait=[w], on_update=[])
        nc.all_engine_barrier()
        nc.sync.drain()
        assert self.sems is not None
        popped = nc._tile_sem_poison_stack.pop()
        assert popped is self._sem_poison
        # clear_and_free_semaphores would range-clear every ALLOCATED sem id
        # (~200+), which walrus lowers to one op per id (~7us of tail).
        # Only ids that appear in the final instruction stream can be
        # non-zero, so hardware-clear just those; do the allocator
        # bookkeeping for the full set.
        allocated = list(self.sems.allocated().values())
        sem_nums = [
            s.num if hasattr(s, "num") else int(s) for s in allocated
        ]
        used = set()
        for fn in nc.m.functions:
            for blk in fn.blocks:
                for inst in blk.instructions:
                    si = inst.sync_info
                    if si is not None:
                        for w in si.on_wait:
                            used.add(w.id)
                        for u in si.on_update:
                            used.add(u.id)
        # hardware sem clears skipped: the walrus NEFF epilogue already
        # restores every semaphore on every engine (observed as ~55
        # EVENT_SEMAPHOREs per engine after the Drain)
        nc._state.prepend_free_semaphores(sem_nums)
        for poison_set in nc._tile_sem_poison_stack:
            poison_set.update(sem_nums)
        # the trailing all_engine_barrier is skipped: nothing after the
        # clear touches semaphores, and the runtime serializes executions


def _split_multi_waits(nc, maxw=1):
    """This walrus build rejects instructions carrying more than one sync
    wait ("Too many sync wait commands"). Move excess waits onto same-engine
    NoOps inserted just before the instruction: sem-ge waits are monotonic
    within the kernel, so waiting for them earlier on the same engine is
    equivalent. sem-eq waits stay on the original instruction."""
    for fn in nc.m.functions:
        for blk in fn.blocks:
            insts = blk.instructions
            if not any(
                i.sync_info is not None and len(i.sync_info.on_wait) > maxw
                for i in insts
            ):
                continue
            out = []
            for inst in insts:
                si = inst.sync_info
                if si is not None and len(si.on_wait) > maxw:
                    keep = [w for w in si.on_wait if "eq" in w.wait_mode]
                    movable = [w for w in si.on_wait if "eq" not in w.wait_mode]
                    while len(keep) < maxw and movable:
                        keep.append(movable.pop(0))
                    assert len(keep) <= maxw, (
                        f"{inst.name}: {len(keep)} non-splittable waits"
                    )
                    for w in movable:
                        nop = mybir.InstNoOp(
                            name=nc.get_next_instruction_name(), ins=[], outs=[]
                        )
                        nop.engine = inst.engine
                        nop.sync_info = mybir.SyncInfo(on_wait=[w], on_update=[])
                        out.append(nop)
                    si.on_wait = keep
                out.append(inst)
            blk.instructions = out


def _gp_pool_avg(nc, out_ap, in_ap):
    """avg-pool the innermost free axis to 1, on the gpsimd engine (InstPool
    lives in the default 'standard' ucode library). bass only exposes pool()
    on the vector engine; InstPool wants a 5-d input AP, expressed via unit
    dims so symbolic re-lowering preserves it."""
    in5 = in_ap.rearrange("p (a b c f) -> p a b c f", a=1, b=1, c=1)
    return nc.gpsimd.add_instruction(
        mybir.InstPool(
            name=nc.get_next_instruction_name(),
            func=mybir.PoolFunctionType.avg,
            ins=[nc.gpsimd.lower_ap(in5, opt=False)],
            outs=[nc.gpsimd.lower_ap(out_ap)],
        )
    )


def _build(general: bool):
    """general=False assumes bq == 0 (rowsum = plain sum of exp via the
    activation's accum_out). general=True weights both reductions with
    phi = exp(v*SCALE) rows so arbitrary biases still work."""
    nc = bass.Bass("TRN2", target_bir_lowering=False, debug=False)

    rt = nc.dram_tensor("rt", (P, NQ, NN), FP8, kind="ExternalInput")
    am = nc.dram_tensor("am", (P, NQ, DD), FP8, kind="ExternalInput")
    ub = nc.dram_tensor("ub", (1, NN), BF16, kind="ExternalInput")
    if general:
        wb = nc.dram_tensor("wb", (1, NN), BF16, kind="ExternalInput")
    out = nc.dram_tensor("out", (2, P, NC), F32, kind="ExternalOutput")

    Exp = mybir.ActivationFunctionType.Exp
    Mult = mybir.AluOpType.mult

    with _TileContext(nc) as tc:
        with (
            tc.tile_pool(name="const", bufs=1) as cpool,
            tc.tile_pool(name="et", bufs=3) as et_pool,
        ):
            # ACT exp-table pre-warm at t=0 so the ~2.7us table load
            # overlaps the input DMA instead of stalling the first exp.
            warm = cpool.tile([1, 1], F32)
            nc.vector.memset(warm[:], 0.0)
            nc.scalar.activation(warm[:], warm[:], Exp, bias=0.0, scale=0.0)

            rt_sb = cpool.tile([P, NQ, NN], FP8, name="rt")
            am_sb = cpool.tile([P, NQ, DD], FP8, name="am")
            ub_row = cpool.tile([1, NN], BF16, name="ubrow")
            ub_sb = cpool.tile([P, NN], BF16, name="ub")
            ones_sb = cpool.tile([1, P], BF16, name="ones")
            nc.vector.memset(ones_sb[:], 1.0)
            if general:
                wb_row = cpool.tile([1, NN], BF16, name="wbrow")
                wb_sb = cpool.tile([P, NN], BF16, name="wb")
            ct_sb = cpool.tile([P, NQ, NN], FP8, name="ct")
            sa_cols = cpool.tile([P, NC], F32, name="sacols")
            if general:
                ra_cols = cpool.tile([P, NC], F32, name="racols")
            else:
                rs_cols = cpool.tile([P, NC], F32, name="rscols")

            # warm-up matmul operands (values irrelevant)
            wmw = cpool.tile([P, 2], FP8, name="wmw")
            wmm = cpool.tile([P, 256], FP8, name="wmm")
            nc.vector.memset(wmw[:], 0.0)
            nc.vector.memset(wmm[:], 0.0)

            # rt on the fast HWDGE queue (needed first), am + u rows on
            # the SWDGE queue.
            # three parallel DMA queues (~165 GB/s each): rt split over the
            # SP and ACT hardware queues, am + u rows on the SWDGE queue
            nc.gpsimd.dma_start(ub_row[:], ub.ap())
            if general:
                nc.gpsimd.dma_start(wb_row[:], wb.ap())
            nc.gpsimd.dma_start(am_sb[:], am.ap())
            nc.sync.dma_start(rt_sb[:, 0:1, :], rt.ap()[:, 0:1, :])
            nc.scalar.dma_start(rt_sb[:, 1:2, :], rt.ap()[:, 1:2, :])
            nc.sync.dma_start(rt_sb[:, 2:3, :], rt.ap()[:, 2:3, :])
            nc.scalar.dma_start(rt_sb[:, 3:4, :], rt.ap()[:, 3:4, :])

            ps = tc.alloc_tile_pool(name="ps", bufs=2, space="PSUM")

            # ---- PE warm-up: lift the HAM clock gate during DMA wait ----
            wt = ps.tile([P, NN], F32, tag="g", name="g")
            for _ in range(N_WARM):
                nc.tensor.matmul(
                    wt[0:2, 0:256], wmw[:], wmm[:],
                    start=True, stop=True, skip_group_check=True,
                )

            # ---- broadcast u (and phi) across partitions: ones^T @ u_row ----
            def bcast(row_tile, dst_sb):
                bt = ps.tile([P, NN], F32, tag="g", name="g")
                for ns in range(NS):
                    nc.tensor.matmul(
                        bt[:, ns * S : (ns + 1) * S],
                        ones_sb[0:1, :],
                        row_tile[0:1, ns * S : (ns + 1) * S],
                        start=True,
                        stop=True,
                    )
                for ns in range(NS):
                    if ns % 2 == 0:
                        nc.vector.tensor_copy(
                            dst_sb[:, ns * S : (ns + 1) * S],
                            bt[:, ns * S : (ns + 1) * S],
                        )
                    else:
                        nc.scalar.copy(
                            dst_sb[:, ns * S : (ns + 1) * S],
                            bt[:, ns * S : (ns + 1) * S],
                        )

            bcast(ub_row, ub_sb)
            if general:
                bcast(wb_row, wb_sb)

            # ---- phase A: ct[q, n] = (R A*32)[n, q] in fp8 ----
            # Two qo tiles live at a time, emitted in DMA-arrival order
            # (all j=0 work needs only rt chunks 0-1; j=1 needs 2-3), which
            # matches what the tile scheduler would reorder to anyway and
            # lets each tile's casts fire a full group before its slot is
            # reused.
            cast_engines = [
                lambda d, s_: nc.vector.tensor_copy(d, s_),
                lambda d, s_: nc.scalar.copy(d, s_),
            ]
            cast_idx = 0

            def a_mms(pt, qo, j):
                for ns in range(NS):
                    nc.tensor.matmul(
                        pt[:, ns * S : (ns + 1) * S],
                        am_sb[:, 2 * j : 2 * j + 2, qo * P : (qo + 1) * P],
                        rt_sb[:, 2 * j : 2 * j + 2, ns * S : (ns + 1) * S],
                        start=(j == 0),
                        stop=(j == NQ // 2 - 1),
                        perf_mode=DR,
                    )

            def a_casts(pt, qo):
                nonlocal cast_idx
                for ns in range(NS):
                    cast_engines[cast_idx % 2](
                        ct_sb[:, qo, ns * S : (ns + 1) * S],
                        pt[:, ns * S : (ns + 1) * S],
                    )
                    cast_idx += 1

            for qa in (0, 2):
                pta = ps.tile([P, NN], F32, tag="g", name="g")
                ptb = ps.tile([P, NN], F32, tag="g", name="g")
                a_mms(pta, qa, 0)
                a_mms(ptb, qa + 1, 0)
                a_mms(pta, qa, 1)
                a_casts(pta, qa)
                a_mms(ptb, qa + 1, 1)
                a_casts(ptb, qa + 1)

            # ---- phase B: exp + reductions per 128-query chunk ----
            for nch in range(NC):
                gt = ps.tile([P, NN], F32, tag="g", name="g")
                for j in range(NQ // 2):
                    for ms in range(NS):
                        nc.tensor.matmul(
                            gt[:, ms * S : (ms + 1) * S],
                            ct_sb[:, 2 * j : 2 * j + 2, nch * P : (nch + 1) * P],
                            rt_sb[:, 2 * j : 2 * j + 2, ms * S : (ms + 1) * S],
                            start=(j == 0),
                            stop=(j == NQ // 2 - 1),
                            perf_mode=DR,
                        )
                et = et_pool.tile([P, NN], BF16, tag="et", name="et")
                nc.scalar.activation(
                    et[:],
                    gt[:],
                    Exp,
                    bias=0.0,
                    scale=SCALE / ASCALE,
                    accum_out=None if general else rs_cols[:, nch : nch + 1],
                )
                if general:
                    # fallback: both reductions whole on DVE
                    et2 = et_pool.tile([P, NN], BF16, tag="et2", name="et2")
                    nc.vector.scalar_tensor_tensor(
                        out=et2[:],
                        in0=et[:],
                        scalar=1.0,
                        in1=wb_sb[:],
                        op0=Mult,
                        op1=Mult,
                        accum_out=ra_cols[:, nch : nch + 1],
                    )
                    nc.vector.scalar_tensor_tensor(
                        out=et[:],
                        in0=et[:],
                        scalar=1.0,
                        in1=ub_sb[:],
                        op0=Mult,
                        op1=Mult,
                        accum_out=sa_cols[:, nch : nch + 1],
                    )
                else:
                    # weighted row-sum fused on DVE
                    nc.vector.scalar_tensor_tensor(
                        out=et[:],
                        in0=et[:],
                        scalar=1.0,
                        in1=ub_sb[:],
                        op0=Mult,
                        op1=Mult,
                        accum_out=sa_cols[:, nch : nch + 1],
                    )

            nc.sync.dma_start(out.ap()[0], sa_cols[:])
            nc.sync.dma_start(
                out.ap()[1], ra_cols[:] if general else rs_cols[:]
            )
            ps.release()

    _split_multi_waits(nc)
    return nc


_NC = {}


def _get_nc(general: bool):
    if general not in _NC:
        _NC[general] = _build(general)
    return _NC[general]


def _host_prep(R, Wq, bq, Wk, bk, Wv, bv, W1, b1, W2, b2):
    """Host-side collapses in fp64. Returns (general, per-core input maps,
    const)."""
    c = W1.T @ W2[0]                       # [512]
    const = float(W2[0] @ b1 + b2[0])
    A = Wq.T @ Wk                          # gamma~ = R A R^T
    general = bool(np.any(bq != 0.0))

    a_h = np.ascontiguousarray(
        (A * ASCALE).reshape(NQ, P, DD).transpose(1, 0, 2)
    ).astype(F8)                           # [128, 4, 512]

    in_maps = []
    for b in range(NB):
        Rb = R[b].astype(np.float64)
        rt_h = np.ascontiguousarray(
            Rb.T.reshape(NQ, P, NN).transpose(1, 0, 2)
        ).astype(F8)                       # [128, 4, 2048]
        u = Rb @ (Wv.T @ c) + float(bv @ c)            # [2048]
        m = {"rt": rt_h, "am": a_h}
        if general:
            phi = np.exp((Rb @ (Wk.T @ bq)) * SCALE)   # per-key weight
            m["ub"] = (phi * u).astype(BF).reshape(1, NN)
            m["wb"] = phi.astype(BF).reshape(1, NN)
        else:
            m["ub"] = u.astype(BF).reshape(1, NN)
        in_maps.append(m)
    return general, in_maps, const


def kernel(R, Wq, bq, Wk, bk, Wv, bv, W1, b1, W2, b2):
    R = np.asarray(R, np.float32)
    args = [np.asarray(x, np.float64) for x in (Wq, bq, Wk, bk, Wv, bv, W1, b1, W2, b2)]
    general, in_maps, const = _host_prep(R, *args)

    nc = _get_nc(general)
    res = run_bass_kernel_spmd(nc, in_maps, core_ids=list(range(N_CORES)))
    outs = np.stack([res.results[b]["out"] for b in range(NB)])  # [8,2,128,16]
    s = outs[:, 0].transpose(0, 2, 1).reshape(NB, NN)
    r = outs[:, 1].transpose(0, 2, 1).reshape(NB, NN)
    return (s / r + np.float32(const)).astype(np.float32)


# revision 13
# speedup vs baseline: 1.0406x; 1.0406x over previous
"""CAAN kernel for Trainium2, 8-core data-parallel (one batch row per core).

Math: the reference is
    Q = R Wq^T + bq ; K = R Wk^T + bk ; V = R Wv^T + bv
    E = exp(Q K^T / sqrt(512)) ; saat = E / rowsum(E)
    winner = (saat V) W1^T W2^T + (W2 b1 + b2)

Algebraic collapses (host, fp64):
1. The W1/W2 head is linear, so with c = W1^T W2[0]:
       winner[n] = (sum_m E[n,m] u[m]) / (sum_m E[n,m]) + const,
   u = V c = R (Wv^T c) + bv.c — a per-asset scalar.
2. gamma = Q K^T = R A R^T + t[n] + v[m] + bq.bk with A = Wq^T Wk.
   The per-n term t cancels in the s/rowsum ratio; when bq == 0 (true
   for this reference) v and bq.bk vanish, leaving gamma~ = R A R^T.

Device ("E layout": query index n on partitions, key index m free):
  warm-up: dummy matmuls during the input DMA so the PE HAM clock-gate
           reaches 8/8 before real work; a dummy exp preloads the ACT
           table set.
  phase A: ct[q, n] = (R A*32)[n, q] via fp8e4 DoubleRow matmuls
           (contraction 256/MM), per-512-slice PSUM->fp8 casts
           alternating DVE/Pool so the 2-slot PSUM rotation never
           stalls on a cast.
  phase B: per 128-query chunk: one 4-bank [128, 2048] PSUM tile of
           gamma~ (8 DR matmuls), ONE Exp activation over all 2048
           columns with accum_out = rowsum for free, then the weighted
           row-sum s[n] via scalar_tensor_tensor (mult + accum) split
           half on DVE, half on Pool.
  out: s halves and rowsum columns [128, 16] f32; host adds halves and
       does winner = s/rowsum + const.

fp8: A pre-scaled by 32 clears the e4m3 denormal floor (entries ~0.016);
the inverse rides the exp scale. Measured rel err ~3e-3 (tol 2e-2).
"""

import math

import ml_dtypes
import numpy as np

import concourse.ap_utils as ap_utils
import concourse.bass as bass
import concourse.mybir as mybir
import concourse.tile as tile
from concourse.bass_utils import run_bass_kernel_spmd
from concourse.vector_clock import ScopedClock


N_CORES = 8
NB, NN, DD = 8, 2048, 512  # batch, assets, feature dim
P = 128
NQ = DD // P   # q chunks (contraction)
NC = NN // P   # n chunks (query rows)
S = 512        # matmul moving free dim / PSUM bank width
NS = NN // S   # slices of 512 along the free axis
HALF = NN // 2
BF16 = mybir.dt.bfloat16
FP8 = mybir.dt.float8e4
F32 = mybir.dt.float32
SCALE = 1.0 / math.sqrt(float(DD))
ASCALE = 32.0
N_WARM = 14    # dummy matmuls to lift the HAM clock gate before phase A
QS = 1536      # DVE handles et[:, :QS]; gpsimd reduces the rest
BF = ml_dtypes.bfloat16
F8 = ml_dtypes.float8_e4m3
DR = mybir.MatmulPerfMode.DoubleRow


class _TileContext(tile.TileContext):
    """Workaround for walrus rejecting >1 sem wait on the kernel-tail Drain
    ("Too many sync wait commands"): put each final wait on its own SP NoOp
    ahead of an unwaited Drain."""

    def _drain_and_barrier(self, tick_clock, wait_clock):
        nc = self.nc
        probe = nc.sync.nop(nofuse=True)
        wait_clock.add_sem_waits(
            probe.ins, ScopedClock({None: tick_clock.global_clock})
        )
        si = probe.ins.sync_info
        waits = list(si.on_wait) if si is not None else []
        if si is not None:
            si.on_wait = []
        # spread the final waits round-robin over all engines so they
        # resolve in parallel; the barrier then guarantees every wait has
        # been observed before the SP drain runs.
        engines = [nc.sync, nc.vector, nc.scalar, nc.tensor, nc.gpsimd]
        for i, w in enumerate(waits):
            n = engines[i % len(engines)].nop(nofuse=True)
            n.ins.sync_info = mybir.SyncInfo(on_wait=[w], on_update=[])
        nc.all_engine_barrier()
        nc.sync.drain()
        assert self.sems is not None
        popped = nc._tile_sem_poison_stack.pop()
        assert popped is self._sem_poison
        # clear_and_free_semaphores would range-clear every ALLOCATED sem id
        # (~200+), which walrus lowers to one op per id (~7us of tail).
        # Only ids that appear in the final instruction stream can be
        # non-zero, so hardware-clear just those; do the allocator
        # bookkeeping for the full set.
        allocated = list(self.sems.allocated().values())
        sem_nums = [
            s.num if hasattr(s, "num") else int(s) for s in allocated
        ]
        used = set()
        for fn in nc.m.functions:
            for blk in fn.blocks:
                for inst in blk.instructions:
                    si = inst.sync_info
                    if si is not None:
                        for w in si.on_wait:
                            used.add(w.id)
                        for u in si.on_update:
                            used.add(u.id)
        # hardware sem clears skipped: the walrus NEFF epilogue already
        # restores every semaphore on every engine (observed as ~55
        # EVENT_SEMAPHOREs per engine after the Drain)
        nc._state.prepend_free_semaphores(sem_nums)
        for poison_set in nc._tile_sem_poison_stack:
            poison_set.update(sem_nums)
        # the trailing all_engine_barrier is skipped: nothing after the
        # clear touches semaphores, and the runtime serializes executions


def _split_multi_waits(nc, maxw=1):
    """This walrus build rejects instructions carrying more than one sync
    wait ("Too many sync wait commands"). Move excess waits onto same-engine
    NoOps inserted just before the instruction: sem-ge waits are monotonic
    within the kernel, so waiting for them earlier on the same engine is
    equivalent. sem-eq waits stay on the original instruction."""
    for fn in nc.m.functions:
        for blk in fn.blocks:
            insts = blk.instructions
            if not any(
                i.sync_info is not None and len(i.sync_info.on_wait) > maxw
                for i in insts
            ):
                continue
            out = []
            for inst in insts:
                si = inst.sync_info
                if si is not None and len(si.on_wait) > maxw:
                    keep = [w for w in si.on_wait if "eq" in w.wait_mode]
                    movable = [w for w in si.on_wait if "eq" not in w.wait_mode]
                    while len(keep) < maxw and movable:
                        keep.append(movable.pop(0))
                    assert len(keep) <= maxw, (
                        f"{inst.name}: {len(keep)} non-splittable waits"
                    )
                    for w in movable:
                        nop = mybir.InstNoOp(
                            name=nc.get_next_instruction_name(), ins=[], outs=[]
                        )
                        nop.engine = inst.engine
                        nop.sync_info = mybir.SyncInfo(on_wait=[w], on_update=[])
                        out.append(nop)
                    si.on_wait = keep
                out.append(inst)
            blk.instructions = out


def _gp_pool_avg(nc, out_ap, in_ap):
    """avg-pool the innermost free axis to 1, on the gpsimd engine (InstPool
    lives in the default 'standard' ucode library). bass only exposes pool()
    on the vector engine; InstPool wants a 5-d input AP, expressed via unit
    dims so symbolic re-lowering preserves it."""
    in5 = in_ap.rearrange("p (a b c f) -> p a b c f", a=1, b=1, c=1)
    return nc.gpsimd.add_instruction(
        mybir.InstPool(
            name=nc.get_next_instruction_name(),
            func=mybir.PoolFunctionType.avg,
            ins=[nc.gpsimd.lower_ap(in5, opt=False)],
            outs=[nc.gpsimd.lower_ap(out_ap)],
        )
    )


def _build(general: bool):
    """general=False assumes bq == 0 (rowsum = plain sum of exp via the
    activation's accum_out). general=True weights both reductions with
    phi = exp(v*SCALE) rows so arbitrary biases still work."""
    nc = bass.Bass("TRN2", target_bir_lowering=False, debug=False)

    rt = nc.dram_tensor("rt", (P, NQ, NN), FP8, kind="ExternalInput")
    am = nc.dram_tensor("am", (P, NQ, DD), FP8, kind="ExternalInput")
    ub = nc.dram_tensor("ub", (1, NN), BF16, kind="ExternalInput")
    if general:
        wb = nc.dram_tensor("wb", (1, NN), BF16, kind="ExternalInput")
    out = nc.dram_tensor("out", (2, P, NC), F32, kind="ExternalOutput")

    Exp = mybir.ActivationFunctionType.Exp
    Mult = mybir.AluOpType.mult

    with _TileContext(nc) as tc:
        with (
            tc.tile_pool(name="const", bufs=1) as cpool,
            tc.tile_pool(name="et", bufs=3) as et_pool,
        ):
            # ACT exp-table pre-warm at t=0 so the ~2.7us table load
            # overlaps the input DMA instead of stalling the first exp.
            warm = cpool.tile([1, 1], F32)
            nc.vector.memset(warm[:], 0.0)
            nc.scalar.activation(warm[:], warm[:], Exp, bias=0.0, scale=0.0)

            rt_sb = cpool.tile([P, NQ, NN], FP8, name="rt")
            am_sb = cpool.tile([P, NQ, DD], FP8, name="am")
            ub_row = cpool.tile([1, NN], BF16, name="ubrow")
            ub_sb = cpool.tile([P, NN], BF16, name="ub")
            ones_sb = cpool.tile([1, P], BF16, name="ones")
            nc.vector.memset(ones_sb[:], 1.0)
            if general:
                wb_row = cpool.tile([1, NN], BF16, name="wbrow")
                wb_sb = cpool.tile([P, NN], BF16, name="wb")
            ct_sb = cpool.tile([P, NQ, NN], FP8, name="ct")
            sa_cols = cpool.tile([P, NC], F32, name="sacols")
            if general:
                ra_cols = cpool.tile([P, NC], F32, name="racols")
            else:
                rs_cols = cpool.tile([P, NC], F32, name="rscols")

            # warm-up matmul operands (values irrelevant)
            wmw = cpool.tile([P, 2], FP8, name="wmw")
            wmm = cpool.tile([P, 256], FP8, name="wmm")
            nc.vector.memset(wmw[:], 0.0)
            nc.vector.memset(wmm[:], 0.0)

            # rt on the fast HWDGE queue (needed first), am + u rows on
            # the SWDGE queue.
            # three parallel DMA queues (~165 GB/s each): rt split over the
            # SP and ACT hardware queues, am + u rows on the SWDGE queue
            nc.sync.dma_start(am_sb[:], am.ap())
            nc.gpsimd.dma_start(ub_row[:], ub.ap())
            if general:
                nc.gpsimd.dma_start(wb_row[:], wb.ap())
            nc.scalar.dma_start(rt_sb[:, 1:2, :], rt.ap()[:, 1:2, :])
            nc.sync.dma_start(rt_sb[:, 0:1, :], rt.ap()[:, 0:1, :])
            nc.scalar.dma_start(rt_sb[:, 3:4, :], rt.ap()[:, 3:4, :])
            nc.sync.dma_start(rt_sb[:, 2:3, :], rt.ap()[:, 2:3, :])

            ps = tc.alloc_tile_pool(name="ps", bufs=2, space="PSUM")

            # ---- PE warm-up: lift the HAM clock gate during DMA wait ----
            wt = ps.tile([P, NN], F32, tag="g", name="g")
            for _ in range(N_WARM):
                nc.tensor.matmul(
                    wt[0:2, 0:256], wmw[:], wmm[:],
                    start=True, stop=True, skip_group_check=True,
                )

            # ---- phase A: ct[q, n] = (R A*32)[n, q] in fp8 ----
            # Two qo tiles live at a time, emitted in DMA-arrival order
            # (all j=0 work needs only rt chunks 0-1; j=1 needs 2-3), which
            # matches what the tile scheduler would reorder to anyway and
            # lets each tile's casts fire a full group before its slot is
            # reused.
            cast_engines = [
                lambda d, s_: nc.vector.tensor_copy(d, s_),
                lambda d, s_: nc.scalar.copy(d, s_),
            ]
            cast_idx = 0

            def a_mms(pt, qo, j):
                for ns in range(NS):
                    nc.tensor.matmul(
                        pt[:, ns * S : (ns + 1) * S],
                        am_sb[:, 2 * j : 2 * j + 2, qo * P : (qo + 1) * P],
                        rt_sb[:, 2 * j : 2 * j + 2, ns * S : (ns + 1) * S],
                        start=(j == 0),
                        stop=(j == NQ // 2 - 1),
                        perf_mode=DR,
                    )

            def a_casts(pt, qo):
                nonlocal cast_idx
                for ns in range(NS):
                    cast_engines[cast_idx % 2](
                        ct_sb[:, qo, ns * S : (ns + 1) * S],
                        pt[:, ns * S : (ns + 1) * S],
                    )
                    cast_idx += 1

            for qa in (0, 2):
                pta = ps.tile([P, NN], F32, tag="g", name="g")
                ptb = ps.tile([P, NN], F32, tag="g", name="g")
                a_mms(pta, qa, 0)
                a_mms(ptb, qa + 1, 0)
                a_mms(pta, qa, 1)
                a_casts(pta, qa)
                a_mms(ptb, qa + 1, 1)
                a_casts(ptb, qa + 1)

            # ---- broadcast u (and phi) across partitions: ones^T @ u_row.
            # Emitted after phase A so its psum slot use and casts don't
            # block the phase-A pipeline; ub_sb is first needed ~2 chunks
            # into phase B. ----
            def bcast(row_tile, dst_sb):
                bt = ps.tile([P, NN], F32, tag="g", name="g")
                for ns in range(NS):
                    nc.tensor.matmul(
                        bt[:, ns * S : (ns + 1) * S],
                        ones_sb[0:1, :],
                        row_tile[0:1, ns * S : (ns + 1) * S],
                        start=True,
                        stop=True,
                    )
                for ns in range(NS):
                    if ns % 2 == 0:
                        nc.vector.tensor_copy(
                            dst_sb[:, ns * S : (ns + 1) * S],
                            bt[:, ns * S : (ns + 1) * S],
                        )
                    else:
                        nc.scalar.copy(
                            dst_sb[:, ns * S : (ns + 1) * S],
                            bt[:, ns * S : (ns + 1) * S],
                        )

            bcast(ub_row, ub_sb)
            if general:
                bcast(wb_row, wb_sb)

            # ---- phase B: exp + reductions per 128-query chunk ----
            for nch in range(NC):
                gt = ps.tile([P, NN], F32, tag="g", name="g")
                for j in range(NQ // 2):
                    for ms in range(NS):
                        nc.tensor.matmul(
                            gt[:, ms * S : (ms + 1) * S],
                            ct_sb[:, 2 * j : 2 * j + 2, nch * P : (nch + 1) * P],
                            rt_sb[:, 2 * j : 2 * j + 2, ms * S : (ms + 1) * S],
                            start=(j == 0),
                            stop=(j == NQ // 2 - 1),
                            perf_mode=DR,
                        )
                et = et_pool.tile([P, NN], BF16, tag="et", name="et")
                nc.scalar.activation(
                    et[:],
                    gt[:],
                    Exp,
                    bias=0.0,
                    scale=SCALE / ASCALE,
                    accum_out=None if general else rs_cols[:, nch : nch + 1],
                )
                if general:
                    # fallback: both reductions whole on DVE
                    et2 = et_pool.tile([P, NN], BF16, tag="et2", name="et2")
                    nc.vector.scalar_tensor_tensor(
                        out=et2[:],
                        in0=et[:],
                        scalar=1.0,
                        in1=wb_sb[:],
                        op0=Mult,
                        op1=Mult,
                        accum_out=ra_cols[:, nch : nch + 1],
                    )
                    nc.vector.scalar_tensor_tensor(
                        out=et[:],
                        in0=et[:],
                        scalar=1.0,
                        in1=ub_sb[:],
                        op0=Mult,
                        op1=Mult,
                        accum_out=sa_cols[:, nch : nch + 1],
                    )
                else:
                    # weighted row-sum fused on DVE
                    nc.vector.scalar_tensor_tensor(
                        out=et[:],
                        in0=et[:],
                        scalar=1.0,
                        in1=ub_sb[:],
                        op0=Mult,
                        op1=Mult,
                        accum_out=sa_cols[:, nch : nch + 1],
                    )

            nc.sync.dma_start(out.ap()[0], sa_cols[:])
            nc.sync.dma_start(
                out.ap()[1], ra_cols[:] if general else rs_cols[:]
            )
            ps.release()

    _split_multi_waits(nc)
    return nc


_NC = {}


def _get_nc(general: bool):
    if general not in _NC:
        _NC[general] = _build(general)
    return _NC[general]


def _host_prep(R, Wq, bq, Wk, bk, Wv, bv, W1, b1, W2, b2):
    """Host-side collapses in fp64. Returns (general, per-core input maps,
    const)."""
    c = W1.T @ W2[0]                       # [512]
    const = float(W2[0] @ b1 + b2[0])
    A = Wq.T @ Wk                          # gamma~ = R A R^T
    general = bool(np.any(bq != 0.0))

    a_h = np.ascontiguousarray(
        (A * ASCALE).reshape(NQ, P, DD).transpose(1, 0, 2)
    ).astype(F8)                           # [128, 4, 512]

    in_maps = []
    for b in range(NB):
        Rb = R[b].astype(np.float64)
        rt_h = np.ascontiguousarray(
            Rb.T.reshape(NQ, P, NN).transpose(1, 0, 2)
        ).astype(F8)                       # [128, 4, 2048]
        u = Rb @ (Wv.T @ c) + float(bv @ c)            # [2048]
        m = {"rt": rt_h, "am": a_h}
        if general:
            phi = np.exp((Rb @ (Wk.T @ bq)) * SCALE)   # per-key weight
            m["ub"] = (phi * u).astype(BF).reshape(1, NN)
            m["wb"] = phi.astype(BF).reshape(1, NN)
        else:
            m["ub"] = u.astype(BF).reshape(1, NN)
        in_maps.append(m)
    return general, in_maps, const


def kernel(R, Wq, bq, Wk, bk, Wv, bv, W1, b1, W2, b2):
    R = np.asarray(R, np.float32)
    args = [np.asarray(x, np.float64) for x in (Wq, bq, Wk, bk, Wv, bv, W1, b1, W2, b2)]
    general, in_maps, const = _host_prep(R, *args)

    nc = _get_nc(general)
    res = run_bass_kernel_spmd(nc, in_maps, core_ids=list(range(N_CORES)))
    outs = np.stack([res.results[b]["out"] for b in range(NB)])  # [8,2,128,16]
    s = outs[:, 0].transpose(0, 2, 1).reshape(NB, NN)
    r = outs[:, 1].transpose(0, 2, 1).reshape(NB, NN)
    return (s / r + np.float32(const)).astype(np.float32)


# revision 14
# speedup vs baseline: 1.0708x; 1.0290x over previous
"""CAAN kernel for Trainium2, 8-core data-parallel (one batch row per core).

Math: the reference is
    Q = R Wq^T + bq ; K = R Wk^T + bk ; V = R Wv^T + bv
    E = exp(Q K^T / sqrt(512)) ; saat = E / rowsum(E)
    winner = (saat V) W1^T W2^T + (W2 b1 + b2)

Algebraic collapses (host, fp64):
1. The W1/W2 head is linear, so with c = W1^T W2[0]:
       winner[n] = (sum_m E[n,m] u[m]) / (sum_m E[n,m]) + const,
   u = V c = R (Wv^T c) + bv.c — a per-asset scalar.
2. gamma = Q K^T = R A R^T + t[n] + v[m] + bq.bk with A = Wq^T Wk.
   The per-n term t cancels in the s/rowsum ratio; when bq == 0 (true
   for this reference) v and bq.bk vanish, leaving gamma~ = R A R^T.

Device ("E layout": query index n on partitions, key index m free):
  warm-up: dummy matmuls during the input DMA so the PE HAM clock-gate
           reaches 8/8 before real work; a dummy exp preloads the ACT
           table set.
  phase A: ct[q, n] = (R A*32)[n, q] via fp8e4 DoubleRow matmuls
           (contraction 256/MM), per-512-slice PSUM->fp8 casts
           alternating DVE/Pool so the 2-slot PSUM rotation never
           stalls on a cast.
  phase B: per 128-query chunk: one 4-bank [128, 2048] PSUM tile of
           gamma~ (8 DR matmuls), ONE Exp activation over all 2048
           columns with accum_out = rowsum for free, then the weighted
           row-sum s[n] via scalar_tensor_tensor (mult + accum) split
           half on DVE, half on Pool.
  out: s halves and rowsum columns [128, 16] f32; host adds halves and
       does winner = s/rowsum + const.

fp8: A pre-scaled by 32 clears the e4m3 denormal floor (entries ~0.016);
the inverse rides the exp scale. Measured rel err ~3e-3 (tol 2e-2).
"""

import math

import ml_dtypes
import numpy as np

import concourse.ap_utils as ap_utils
import concourse.bass as bass
import concourse.mybir as mybir
import concourse.tile as tile
from concourse.bass_utils import run_bass_kernel_spmd
from concourse.vector_clock import ScopedClock


N_CORES = 8
NB, NN, DD = 8, 2048, 512  # batch, assets, feature dim
P = 128
NQ = DD // P   # q chunks (contraction)
NC = NN // P   # n chunks (query rows)
S = 512        # matmul moving free dim / PSUM bank width
NS = NN // S   # slices of 512 along the free axis
HALF = NN // 2
BF16 = mybir.dt.bfloat16
FP8 = mybir.dt.float8e4
F32 = mybir.dt.float32
SCALE = 1.0 / math.sqrt(float(DD))
ASCALE = 32.0
N_WARM = 14    # dummy matmuls to lift the HAM clock gate before phase A
QS = 1536      # DVE handles et[:, :QS]; gpsimd reduces the rest
BF = ml_dtypes.bfloat16
F8 = ml_dtypes.float8_e4m3
DR = mybir.MatmulPerfMode.DoubleRow


class _TileContext(tile.TileContext):
    """Workaround for walrus rejecting >1 sem wait on the kernel-tail Drain
    ("Too many sync wait commands"): put each final wait on its own SP NoOp
    ahead of an unwaited Drain."""

    def _drain_and_barrier(self, tick_clock, wait_clock):
        nc = self.nc
        probe = nc.sync.nop(nofuse=True)
        wait_clock.add_sem_waits(
            probe.ins, ScopedClock({None: tick_clock.global_clock})
        )
        si = probe.ins.sync_info
        waits = list(si.on_wait) if si is not None else []
        if si is not None:
            si.on_wait = []
        # spread the final waits round-robin over all engines so they
        # resolve in parallel; the barrier then guarantees every wait has
        # been observed before the SP drain runs.
        engines = [nc.sync, nc.vector, nc.scalar, nc.tensor, nc.gpsimd]
        for i, w in enumerate(waits):
            n = engines[i % len(engines)].nop(nofuse=True)
            n.ins.sync_info = mybir.SyncInfo(on_wait=[w], on_update=[])
        nc.all_engine_barrier()
        nc.sync.drain()
        assert self.sems is not None
        popped = nc._tile_sem_poison_stack.pop()
        assert popped is self._sem_poison
        # clear_and_free_semaphores would range-clear every ALLOCATED sem id
        # (~200+), which walrus lowers to one op per id (~7us of tail).
        # Only ids that appear in the final instruction stream can be
        # non-zero, so hardware-clear just those; do the allocator
        # bookkeeping for the full set.
        allocated = list(self.sems.allocated().values())
        sem_nums = [
            s.num if hasattr(s, "num") else int(s) for s in allocated
        ]
        used = set()
        for fn in nc.m.functions:
            for blk in fn.blocks:
                for inst in blk.instructions:
                    si = inst.sync_info
                    if si is not None:
                        for w in si.on_wait:
                            used.add(w.id)
                        for u in si.on_update:
                            used.add(u.id)
        # hardware sem clears skipped: the walrus NEFF epilogue already
        # restores every semaphore on every engine (observed as ~55
        # EVENT_SEMAPHOREs per engine after the Drain)
        nc._state.prepend_free_semaphores(sem_nums)
        for poison_set in nc._tile_sem_poison_stack:
            poison_set.update(sem_nums)
        # the trailing all_engine_barrier is skipped: nothing after the
        # clear touches semaphores, and the runtime serializes executions


def _split_multi_waits(nc, maxw=1):
    """This walrus build rejects instructions carrying more than one sync
    wait ("Too many sync wait commands"). Move excess waits onto same-engine
    NoOps inserted just before the instruction: sem-ge waits are monotonic
    within the kernel, so waiting for them earlier on the same engine is
    equivalent. sem-eq waits stay on the original instruction."""
    for fn in nc.m.functions:
        for blk in fn.blocks:
            insts = blk.instructions
            if not any(
                i.sync_info is not None and len(i.sync_info.on_wait) > maxw
                for i in insts
            ):
                continue
            out = []
            for inst in insts:
                si = inst.sync_info
                if si is not None and len(si.on_wait) > maxw:
                    keep = [w for w in si.on_wait if "eq" in w.wait_mode]
                    movable = [w for w in si.on_wait if "eq" not in w.wait_mode]
                    while len(keep) < maxw and movable:
                        keep.append(movable.pop(0))
                    assert len(keep) <= maxw, (
                        f"{inst.name}: {len(keep)} non-splittable waits"
                    )
                    for w in movable:
                        nop = mybir.InstNoOp(
                            name=nc.get_next_instruction_name(), ins=[], outs=[]
                        )
                        nop.engine = inst.engine
                        nop.sync_info = mybir.SyncInfo(on_wait=[w], on_update=[])
                        out.append(nop)
                    si.on_wait = keep
                out.append(inst)
            blk.instructions = out


def _gp_pool_avg(nc, out_ap, in_ap):
    """avg-pool the innermost free axis to 1, on the gpsimd engine (InstPool
    lives in the default 'standard' ucode library). bass only exposes pool()
    on the vector engine; InstPool wants a 5-d input AP, expressed via unit
    dims so symbolic re-lowering preserves it."""
    in5 = in_ap.rearrange("p (a b c f) -> p a b c f", a=1, b=1, c=1)
    return nc.gpsimd.add_instruction(
        mybir.InstPool(
            name=nc.get_next_instruction_name(),
            func=mybir.PoolFunctionType.avg,
            ins=[nc.gpsimd.lower_ap(in5, opt=False)],
            outs=[nc.gpsimd.lower_ap(out_ap)],
        )
    )


def _build(general: bool):
    """general=False assumes bq == 0 (rowsum = plain sum of exp via the
    activation's accum_out). general=True weights both reductions with
    phi = exp(v*SCALE) rows so arbitrary biases still work."""
    nc = bass.Bass("TRN2", target_bir_lowering=False, debug=False)

    rt = nc.dram_tensor("rt", (P, NQ, NN), FP8, kind="ExternalInput")
    am = nc.dram_tensor("am", (P, NQ, DD), FP8, kind="ExternalInput")
    ub = nc.dram_tensor("ub", (1, NN), BF16, kind="ExternalInput")
    if general:
        wb = nc.dram_tensor("wb", (1, NN), BF16, kind="ExternalInput")
    out = nc.dram_tensor("out", (2, P, NC), F32, kind="ExternalOutput")

    Exp = mybir.ActivationFunctionType.Exp
    Mult = mybir.AluOpType.mult

    with _TileContext(nc) as tc:
        with (
            tc.tile_pool(name="const", bufs=1) as cpool,
            tc.tile_pool(name="et", bufs=3) as et_pool,
        ):
            # ACT exp-table pre-warm at t=0 so the ~2.7us table load
            # overlaps the input DMA instead of stalling the first exp.
            warm = cpool.tile([1, 1], F32)
            nc.vector.memset(warm[:], 0.0)
            nc.scalar.activation(warm[:], warm[:], Exp, bias=0.0, scale=0.0)

            rt_sb = cpool.tile([P, NQ, NN], FP8, name="rt")
            am_sb = cpool.tile([P, NQ, DD], FP8, name="am")
            ub_row = cpool.tile([1, NN], BF16, name="ubrow")
            ub_sb = cpool.tile([P, NN], BF16, name="ub")
            ones_sb = cpool.tile([1, P], BF16, name="ones")
            nc.vector.memset(ones_sb[:], 1.0)
            if general:
                wb_row = cpool.tile([1, NN], BF16, name="wbrow")
                wb_sb = cpool.tile([P, NN], BF16, name="wb")
            ct_sb = cpool.tile([P, NQ, NN], FP8, name="ct")
            sa_cols = cpool.tile([P, NC], F32, name="sacols")
            if general:
                ra_cols = cpool.tile([P, NC], F32, name="racols")
            else:
                rs_cols = cpool.tile([P, NC], F32, name="rscols")

            # warm-up matmul operands (values irrelevant)
            wmw = cpool.tile([P, 2], FP8, name="wmw")
            wmm = cpool.tile([P, 256], FP8, name="wmm")
            nc.vector.memset(wmw[:], 0.0)
            nc.vector.memset(wmm[:], 0.0)

            # rt on the fast HWDGE queue (needed first), am + u rows on
            # the SWDGE queue.
            # three parallel DMA queues (~165 GB/s each): rt split over the
            # SP and ACT hardware queues, am + u rows on the SWDGE queue
            nc.sync.dma_start(am_sb[:], am.ap())
            nc.gpsimd.dma_start(ub_row[:], ub.ap())
            if general:
                nc.gpsimd.dma_start(wb_row[:], wb.ap())
            nc.scalar.dma_start(rt_sb[:, 1:2, :], rt.ap()[:, 1:2, :])
            nc.sync.dma_start(rt_sb[:, 0:1, :], rt.ap()[:, 0:1, :])
            nc.scalar.dma_start(rt_sb[:, 3:4, :], rt.ap()[:, 3:4, :])
            nc.sync.dma_start(rt_sb[:, 2:3, :], rt.ap()[:, 2:3, :])

            ps = tc.alloc_tile_pool(name="ps", bufs=2, space="PSUM")

            # ---- PE warm-up: lift the HAM clock gate during DMA wait ----
            wt = ps.tile([P, NN], F32, tag="g", name="g")
            for _ in range(N_WARM):
                nc.tensor.matmul(
                    wt[0:2, 0:256], wmw[:], wmm[:],
                    start=True, stop=True, skip_group_check=True,
                )

            # ---- phase A: ct[q, n] = (R A*32)[n, q] in fp8 ----
            # Two qo tiles live at a time, emitted in DMA-arrival order
            # (all j=0 work needs only rt chunks 0-1; j=1 needs 2-3), which
            # matches what the tile scheduler would reorder to anyway and
            # lets each tile's casts fire a full group before its slot is
            # reused.
            cast_engines = [
                lambda d, s_: nc.vector.tensor_copy(d, s_),
                lambda d, s_: nc.scalar.copy(d, s_),
            ]
            cast_idx = 0

            def a_mms(pt, qo, j, with_casts=False):
                nonlocal cast_idx
                for ns in range(NS):
                    nc.tensor.matmul(
                        pt[:, ns * S : (ns + 1) * S],
                        am_sb[:, 2 * j : 2 * j + 2, qo * P : (qo + 1) * P],
                        rt_sb[:, 2 * j : 2 * j + 2, ns * S : (ns + 1) * S],
                        start=(j == 0),
                        stop=(j == NQ // 2 - 1),
                        perf_mode=DR,
                    )
                    if with_casts:
                        # fire each slice's fp32->fp8 cast the moment its
                        # accumulation group closes
                        cast_engines[cast_idx % 2](
                            ct_sb[:, qo, ns * S : (ns + 1) * S],
                            pt[:, ns * S : (ns + 1) * S],
                        )
                        cast_idx += 1

            for qa in (0, 2):
                pta = ps.tile([P, NN], F32, tag="g", name="g")
                ptb = ps.tile([P, NN], F32, tag="g", name="g")
                a_mms(pta, qa, 0)
                a_mms(ptb, qa + 1, 0)
                a_mms(pta, qa, 1, with_casts=True)
                a_mms(ptb, qa + 1, 1, with_casts=True)

            # ---- broadcast u (and phi) across partitions: ones^T @ u_row.
            # Emitted after phase A so its psum slot use and casts don't
            # block the phase-A pipeline; ub_sb is first needed ~2 chunks
            # into phase B. ----
            def bcast(row_tile, dst_sb):
                bt = ps.tile([P, NN], F32, tag="g", name="g")
                for ns in range(NS):
                    nc.tensor.matmul(
                        bt[:, ns * S : (ns + 1) * S],
                        ones_sb[0:1, :],
                        row_tile[0:1, ns * S : (ns + 1) * S],
                        start=True,
                        stop=True,
                    )
                for ns in range(NS):
                    if ns % 2 == 0:
                        nc.vector.tensor_copy(
                            dst_sb[:, ns * S : (ns + 1) * S],
                            bt[:, ns * S : (ns + 1) * S],
                        )
                    else:
                        nc.scalar.copy(
                            dst_sb[:, ns * S : (ns + 1) * S],
                            bt[:, ns * S : (ns + 1) * S],
                        )

            bcast(ub_row, ub_sb)
            if general:
                bcast(wb_row, wb_sb)

            # ---- phase B: exp + reductions per 128-query chunk ----
            for nch in range(NC):
                gt = ps.tile([P, NN], F32, tag="g", name="g")
                for j in range(NQ // 2):
                    for ms in range(NS):
                        nc.tensor.matmul(
                            gt[:, ms * S : (ms + 1) * S],
                            ct_sb[:, 2 * j : 2 * j + 2, nch * P : (nch + 1) * P],
                            rt_sb[:, 2 * j : 2 * j + 2, ms * S : (ms + 1) * S],
                            start=(j == 0),
                            stop=(j == NQ // 2 - 1),
                            perf_mode=DR,
                        )
                et = et_pool.tile([P, NN], BF16, tag="et", name="et")
                nc.scalar.activation(
                    et[:],
                    gt[:],
                    Exp,
                    bias=0.0,
                    scale=SCALE / ASCALE,
                    accum_out=None if general else rs_cols[:, nch : nch + 1],
                )
                if general:
                    # fallback: both reductions whole on DVE
                    et2 = et_pool.tile([P, NN], BF16, tag="et2", name="et2")
                    nc.vector.scalar_tensor_tensor(
                        out=et2[:],
                        in0=et[:],
                        scalar=1.0,
                        in1=wb_sb[:],
                        op0=Mult,
                        op1=Mult,
                        accum_out=ra_cols[:, nch : nch + 1],
                    )
                    nc.vector.scalar_tensor_tensor(
                        out=et[:],
                        in0=et[:],
                        scalar=1.0,
                        in1=ub_sb[:],
                        op0=Mult,
                        op1=Mult,
                        accum_out=sa_cols[:, nch : nch + 1],
                    )
                else:
                    # weighted row-sum fused on DVE
                    nc.vector.scalar_tensor_tensor(
                        out=et[:],
                        in0=et[:],
                        scalar=1.0,
                        in1=ub_sb[:],
                        op0=Mult,
                        op1=Mult,
                        accum_out=sa_cols[:, nch : nch + 1],
                    )

            nc.sync.dma_start(out.ap()[0], sa_cols[:])
            nc.sync.dma_start(
                out.ap()[1], ra_cols[:] if general else rs_cols[:]
            )
            ps.release()

    _split_multi_waits(nc)
    return nc


_NC = {}


def _get_nc(general: bool):
    if general not in _NC:
        _NC[general] = _build(general)
    return _NC[general]


def _host_prep(R, Wq, bq, Wk, bk, Wv, bv, W1, b1, W2, b2):
    """Host-side collapses in fp64. Returns (general, per-core input maps,
    const)."""
    c = W1.T @ W2[0]                       # [512]
    const = float(W2[0] @ b1 + b2[0])
    A = Wq.T @ Wk                          # gamma~ = R A R^T
    general = bool(np.any(bq != 0.0))

    a_h = np.ascontiguousarray(
        (A * ASCALE).reshape(NQ, P, DD).transpose(1, 0, 2)
    ).astype(F8)                           # [128, 4, 512]

    in_maps = []
    for b in range(NB):
        Rb = R[b].astype(np.float64)
        rt_h = np.ascontiguousarray(
            Rb.T.reshape(NQ, P, NN).transpose(1, 0, 2)
        ).astype(F8)                       # [128, 4, 2048]
        u = Rb @ (Wv.T @ c) + float(bv @ c)            # [2048]
        m = {"rt": rt_h, "am": a_h}
        if general:
            phi = np.exp((Rb @ (Wk.T @ bq)) * SCALE)   # per-key weight
            m["ub"] = (phi * u).astype(BF).reshape(1, NN)
            m["wb"] = phi.astype(BF).reshape(1, NN)
        else:
            m["ub"] = u.astype(BF).reshape(1, NN)
        in_maps.append(m)
    return general, in_maps, const


def kernel(R, Wq, bq, Wk, bk, Wv, bv, W1, b1, W2, b2):
    R = np.asarray(R, np.float32)
    args = [np.asarray(x, np.float64) for x in (Wq, bq, Wk, bk, Wv, bv, W1, b1, W2, b2)]
    general, in_maps, const = _host_prep(R, *args)

    nc = _get_nc(general)
    res = run_bass_kernel_spmd(nc, in_maps, core_ids=list(range(N_CORES)))
    outs = np.stack([res.results[b]["out"] for b in range(NB)])  # [8,2,128,16]
    s = outs[:, 0].transpose(0, 2, 1).reshape(NB, NN)
    r = outs[:, 1].transpose(0, 2, 1).reshape(NB, NN)
    return (s / r + np.float32(const)).astype(np.float32)


# revision 15
# speedup vs baseline: 1.0855x; 1.0138x over previous
"""CAAN kernel for Trainium2, 8-core data-parallel (one batch row per core).

Math: the reference is
    Q = R Wq^T + bq ; K = R Wk^T + bk ; V = R Wv^T + bv
    E = exp(Q K^T / sqrt(512)) ; saat = E / rowsum(E)
    winner = (saat V) W1^T W2^T + (W2 b1 + b2)

Algebraic collapses (host, fp64):
1. The W1/W2 head is linear, so with c = W1^T W2[0]:
       winner[n] = (sum_m E[n,m] u[m]) / (sum_m E[n,m]) + const,
   u = V c = R (Wv^T c) + bv.c — a per-asset scalar.
2. gamma = Q K^T = R A R^T + t[n] + v[m] + bq.bk with A = Wq^T Wk.
   The per-n term t cancels in the s/rowsum ratio; when bq == 0 (true
   for this reference) v and bq.bk vanish, leaving gamma~ = R A R^T.

Device ("E layout": query index n on partitions, key index m free):
  warm-up: dummy matmuls during the input DMA so the PE HAM clock-gate
           reaches 8/8 before real work; a dummy exp preloads the ACT
           table set.
  phase A: ct[q, n] = (R A*32)[n, q] via fp8e4 DoubleRow matmuls
           (contraction 256/MM), per-512-slice PSUM->fp8 casts
           alternating DVE/Pool so the 2-slot PSUM rotation never
           stalls on a cast.
  phase B: per 128-query chunk: one 4-bank [128, 2048] PSUM tile of
           gamma~ (8 DR matmuls), ONE Exp activation over all 2048
           columns with accum_out = rowsum for free, then the weighted
           row-sum s[n] via scalar_tensor_tensor (mult + accum) split
           half on DVE, half on Pool.
  out: s halves and rowsum columns [128, 16] f32; host adds halves and
       does winner = s/rowsum + const.

fp8: A pre-scaled by 32 clears the e4m3 denormal floor (entries ~0.016);
the inverse rides the exp scale. Measured rel err ~3e-3 (tol 2e-2).
"""

import math

import ml_dtypes
import numpy as np

import concourse.ap_utils as ap_utils
import concourse.bass as bass
import concourse.mybir as mybir
import concourse.tile as tile
from concourse.bass_utils import run_bass_kernel_spmd
from concourse.vector_clock import ScopedClock


N_CORES = 8
NB, NN, DD = 8, 2048, 512  # batch, assets, feature dim
P = 128
NQ = DD // P   # q chunks (contraction)
NC = NN // P   # n chunks (query rows)
S = 512        # matmul moving free dim / PSUM bank width
NS = NN // S   # slices of 512 along the free axis
HALF = NN // 2
BF16 = mybir.dt.bfloat16
FP8 = mybir.dt.float8e4
F32 = mybir.dt.float32
SCALE = 1.0 / math.sqrt(float(DD))
ASCALE = 32.0
N_WARM = 14    # dummy matmuls to lift the HAM clock gate before phase A
QS = 1536      # DVE handles et[:, :QS]; gpsimd reduces the rest
BF = ml_dtypes.bfloat16
F8 = ml_dtypes.float8_e4m3
DR = mybir.MatmulPerfMode.DoubleRow


class _TileContext(tile.TileContext):
    """Workaround for walrus rejecting >1 sem wait on the kernel-tail Drain
    ("Too many sync wait commands"): put each final wait on its own SP NoOp
    ahead of an unwaited Drain."""

    def _drain_and_barrier(self, tick_clock, wait_clock):
        nc = self.nc
        probe = nc.sync.nop(nofuse=True)
        wait_clock.add_sem_waits(
            probe.ins, ScopedClock({None: tick_clock.global_clock})
        )
        si = probe.ins.sync_info
        waits = list(si.on_wait) if si is not None else []
        if si is not None:
            si.on_wait = []
        # spread the final waits round-robin over all engines so they
        # resolve in parallel; the barrier then guarantees every wait has
        # been observed before the SP drain runs.
        engines = [nc.sync, nc.vector, nc.scalar, nc.tensor, nc.gpsimd]
        for i, w in enumerate(waits):
            n = engines[i % len(engines)].nop(nofuse=True)
            n.ins.sync_info = mybir.SyncInfo(on_wait=[w], on_update=[])
        nc.all_engine_barrier()
        nc.sync.drain()
        assert self.sems is not None
        popped = nc._tile_sem_poison_stack.pop()
        assert popped is self._sem_poison
        # clear_and_free_semaphores would range-clear every ALLOCATED sem id
        # (~200+), which walrus lowers to one op per id (~7us of tail).
        # Only ids that appear in the final instruction stream can be
        # non-zero, so hardware-clear just those; do the allocator
        # bookkeeping for the full set.
        allocated = list(self.sems.allocated().values())
        sem_nums = [
            s.num if hasattr(s, "num") else int(s) for s in allocated
        ]
        used = set()
        for fn in nc.m.functions:
            for blk in fn.blocks:
                for inst in blk.instructions:
                    si = inst.sync_info
                    if si is not None:
                        for w in si.on_wait:
                            used.add(w.id)
                        for u in si.on_update:
                            used.add(u.id)
        # hardware sem clears skipped: the walrus NEFF epilogue already
        # restores every semaphore on every engine (observed as ~55
        # EVENT_SEMAPHOREs per engine after the Drain)
        nc._state.prepend_free_semaphores(sem_nums)
        for poison_set in nc._tile_sem_poison_stack:
            poison_set.update(sem_nums)
        # the trailing all_engine_barrier is skipped: nothing after the
        # clear touches semaphores, and the runtime serializes executions


def _split_multi_waits(nc, maxw=1):
    """This walrus build rejects instructions carrying more than one sync
    wait ("Too many sync wait commands"). Move excess waits onto same-engine
    NoOps inserted just before the instruction: sem-ge waits are monotonic
    within the kernel, so waiting for them earlier on the same engine is
    equivalent. sem-eq waits stay on the original instruction."""
    for fn in nc.m.functions:
        for blk in fn.blocks:
            insts = blk.instructions
            if not any(
                i.sync_info is not None and len(i.sync_info.on_wait) > maxw
                for i in insts
            ):
                continue
            out = []
            for inst in insts:
                si = inst.sync_info
                if si is not None and len(si.on_wait) > maxw:
                    keep = [w for w in si.on_wait if "eq" in w.wait_mode]
                    movable = [w for w in si.on_wait if "eq" not in w.wait_mode]
                    while len(keep) < maxw and movable:
                        keep.append(movable.pop(0))
                    assert len(keep) <= maxw, (
                        f"{inst.name}: {len(keep)} non-splittable waits"
                    )
                    for w in movable:
                        nop = mybir.InstNoOp(
                            name=nc.get_next_instruction_name(), ins=[], outs=[]
                        )
                        nop.engine = inst.engine
                        nop.sync_info = mybir.SyncInfo(on_wait=[w], on_update=[])
                        out.append(nop)
                    si.on_wait = keep
                out.append(inst)
            blk.instructions = out


def _gp_pool_avg(nc, out_ap, in_ap):
    """avg-pool the innermost free axis to 1, on the gpsimd engine (InstPool
    lives in the default 'standard' ucode library). bass only exposes pool()
    on the vector engine; InstPool wants a 5-d input AP, expressed via unit
    dims so symbolic re-lowering preserves it."""
    in5 = in_ap.rearrange("p (a b c f) -> p a b c f", a=1, b=1, c=1)
    return nc.gpsimd.add_instruction(
        mybir.InstPool(
            name=nc.get_next_instruction_name(),
            func=mybir.PoolFunctionType.avg,
            ins=[nc.gpsimd.lower_ap(in5, opt=False)],
            outs=[nc.gpsimd.lower_ap(out_ap)],
        )
    )


def _build(general: bool):
    """general=False assumes bq == 0 (rowsum = plain sum of exp via the
    activation's accum_out). general=True weights both reductions with
    phi = exp(v*SCALE) rows so arbitrary biases still work."""
    nc = bass.Bass("TRN2", target_bir_lowering=False, debug=False)

    rt = nc.dram_tensor("rt", (P, NQ, NN), FP8, kind="ExternalInput")
    am = nc.dram_tensor("am", (P, NQ, DD), FP8, kind="ExternalInput")
    ub = nc.dram_tensor("ub", (1, NN), BF16, kind="ExternalInput")
    if general:
        wb = nc.dram_tensor("wb", (1, NN), BF16, kind="ExternalInput")
    out = nc.dram_tensor("out", (2, P, NC), F32, kind="ExternalOutput")

    Exp = mybir.ActivationFunctionType.Exp
    Mult = mybir.AluOpType.mult

    with _TileContext(nc) as tc:
        with (
            tc.tile_pool(name="const", bufs=1) as cpool,
            tc.tile_pool(name="et", bufs=3) as et_pool,
        ):
            # ACT exp-table pre-warm at t=0 so the ~2.7us table load
            # overlaps the input DMA instead of stalling the first exp.
            warm = cpool.tile([1, 1], F32)
            nc.vector.memset(warm[:], 0.0)
            nc.scalar.activation(warm[:], warm[:], Exp, bias=0.0, scale=0.0)

            rt_sb = cpool.tile([P, NQ, NN], FP8, name="rt")
            am_sb = cpool.tile([P, NQ, DD], FP8, name="am")
            ub_row = cpool.tile([1, NN], BF16, name="ubrow")
            ub_sb = cpool.tile([P, NN], BF16, name="ub")
            ones_sb = cpool.tile([1, P], BF16, name="ones")
            nc.vector.memset(ones_sb[:], 1.0)
            if general:
                wb_row = cpool.tile([1, NN], BF16, name="wbrow")
                wb_sb = cpool.tile([P, NN], BF16, name="wb")
            ct_sb = cpool.tile([P, NQ, NN], FP8, name="ct")
            sa_cols = cpool.tile([P, NC], F32, name="sacols")
            if general:
                ra_cols = cpool.tile([P, NC], F32, name="racols")
            else:
                rs_cols = cpool.tile([P, NC], F32, name="rscols")

            # warm-up matmul operands (values irrelevant)
            wmw = cpool.tile([P, 2], FP8, name="wmw")
            wmm = cpool.tile([P, 256], FP8, name="wmm")
            nc.vector.memset(wmw[:], 0.0)
            nc.vector.memset(wmm[:], 0.0)

            # rt on the fast HWDGE queue (needed first), am + u rows on
            # the SWDGE queue.
            # three parallel DMA queues (~165 GB/s each): rt split over the
            # SP and ACT hardware queues, am + u rows on the SWDGE queue
            nc.sync.dma_start(am_sb[:], am.ap())
            nc.gpsimd.dma_start(ub_row[:], ub.ap())
            if general:
                nc.gpsimd.dma_start(wb_row[:], wb.ap())
            nc.scalar.dma_start(rt_sb[:, 1:2, :], rt.ap()[:, 1:2, :])
            nc.sync.dma_start(rt_sb[:, 0:1, :], rt.ap()[:, 0:1, :])
            nc.scalar.dma_start(rt_sb[:, 3:4, :], rt.ap()[:, 3:4, :])
            nc.sync.dma_start(rt_sb[:, 2:3, :], rt.ap()[:, 2:3, :])

            ps = tc.alloc_tile_pool(name="ps", bufs=2, space="PSUM")

            # ---- PE warm-up: lift the HAM clock gate during DMA wait ----
            wt = ps.tile([P, NN], F32, tag="g", name="g")
            for _ in range(N_WARM):
                nc.tensor.matmul(
                    wt[0:2, 0:256], wmw[:], wmm[:],
                    start=True, stop=True, skip_group_check=True,
                )

            # ---- phase A: ct[q, n] = (R A*32)[n, q] in fp8 ----
            # Two qo tiles live at a time, emitted in DMA-arrival order
            # (all j=0 work needs only rt chunks 0-1; j=1 needs 2-3), which
            # matches what the tile scheduler would reorder to anyway and
            # lets each tile's casts fire a full group before its slot is
            # reused.
            cast_engines = [
                lambda d, s_: nc.vector.tensor_copy(d, s_),
                lambda d, s_: nc.scalar.copy(d, s_),
            ]
            cast_idx = 0

            def a_mms(pt, qo, j):
                for ns in range(NS):
                    nc.tensor.matmul(
                        pt[:, ns * S : (ns + 1) * S],
                        am_sb[:, 2 * j : 2 * j + 2, qo * P : (qo + 1) * P],
                        rt_sb[:, 2 * j : 2 * j + 2, ns * S : (ns + 1) * S],
                        start=(j == 0),
                        stop=(j == NQ // 2 - 1),
                        perf_mode=DR,
                    )

            for qa in (0, 2):
                pta = ps.tile([P, NN], F32, tag="g", name="g")
                ptb = ps.tile([P, NN], F32, tag="g", name="g")
                a_mms(pta, qa, 0)
                a_mms(ptb, qa + 1, 0)
                a_mms(pta, qa, 1)
                # ONE whole-tile cast per qo: PSUM tile deps are whole-tile,
                # so per-slice casts would serialize against later matmuls
                # on the same tile; a single cast avoids the WAR chain.
                cast_engines[cast_idx % 2](ct_sb[:, qa, :], pta[:])
                cast_idx += 1
                a_mms(ptb, qa + 1, 1)
                cast_engines[cast_idx % 2](ct_sb[:, qa + 1, :], ptb[:])
                cast_idx += 1

            # ---- broadcast u (and phi) across partitions: ones^T @ u_row.
            # Emitted after phase A so its psum slot use and casts don't
            # block the phase-A pipeline; ub_sb is first needed ~2 chunks
            # into phase B. ----
            def bcast(row_tile, dst_sb):
                bt = ps.tile([P, NN], F32, tag="g", name="g")
                for ns in range(NS):
                    nc.tensor.matmul(
                        bt[:, ns * S : (ns + 1) * S],
                        ones_sb[0:1, :],
                        row_tile[0:1, ns * S : (ns + 1) * S],
                        start=True,
                        stop=True,
                    )
                for ns in range(NS):
                    if ns % 2 == 0:
                        nc.vector.tensor_copy(
                            dst_sb[:, ns * S : (ns + 1) * S],
                            bt[:, ns * S : (ns + 1) * S],
                        )
                    else:
                        nc.scalar.copy(
                            dst_sb[:, ns * S : (ns + 1) * S],
                            bt[:, ns * S : (ns + 1) * S],
                        )

            bcast(ub_row, ub_sb)
            if general:
                bcast(wb_row, wb_sb)

            # ---- phase B: exp + reductions per 128-query chunk ----
            for nch in range(NC):
                gt = ps.tile([P, NN], F32, tag="g", name="g")
                for j in range(NQ // 2):
                    for ms in range(NS):
                        nc.tensor.matmul(
                            gt[:, ms * S : (ms + 1) * S],
                            ct_sb[:, 2 * j : 2 * j + 2, nch * P : (nch + 1) * P],
                            rt_sb[:, 2 * j : 2 * j + 2, ms * S : (ms + 1) * S],
                            start=(j == 0),
                            stop=(j == NQ // 2 - 1),
                            perf_mode=DR,
                        )
                et = et_pool.tile([P, NN], BF16, tag="et", name="et")
                nc.scalar.activation(
                    et[:],
                    gt[:],
                    Exp,
                    bias=0.0,
                    scale=SCALE / ASCALE,
                    accum_out=None if general else rs_cols[:, nch : nch + 1],
                )
                if general:
                    # fallback: both reductions whole on DVE
                    et2 = et_pool.tile([P, NN], BF16, tag="et2", name="et2")
                    nc.vector.scalar_tensor_tensor(
                        out=et2[:],
                        in0=et[:],
                        scalar=1.0,
                        in1=wb_sb[:],
                        op0=Mult,
                        op1=Mult,
                        accum_out=ra_cols[:, nch : nch + 1],
                    )
                    nc.vector.scalar_tensor_tensor(
                        out=et[:],
                        in0=et[:],
                        scalar=1.0,
                        in1=ub_sb[:],
                        op0=Mult,
                        op1=Mult,
                        accum_out=sa_cols[:, nch : nch + 1],
                    )
                else:
                    # weighted row-sum fused on DVE
                    nc.vector.scalar_tensor_tensor(
                        out=et[:],
                        in0=et[:],
                        scalar=1.0,
                        in1=ub_sb[:],
                        op0=Mult,
                        op1=Mult,
                        accum_out=sa_cols[:, nch : nch + 1],
                    )

            nc.sync.dma_start(out.ap()[0], sa_cols[:])
            nc.sync.dma_start(
                out.ap()[1], ra_cols[:] if general else rs_cols[:]
            )
            ps.release()

    _split_multi_waits(nc)
    return nc


_NC = {}


def _get_nc(general: bool):
    if general not in _NC:
        _NC[general] = _build(general)
    return _NC[general]


def _host_prep(R, Wq, bq, Wk, bk, Wv, bv, W1, b1, W2, b2):
    """Host-side collapses in fp64. Returns (general, per-core input maps,
    const)."""
    c = W1.T @ W2[0]                       # [512]
    const = float(W2[0] @ b1 + b2[0])
    A = Wq.T @ Wk                          # gamma~ = R A R^T
    general = bool(np.any(bq != 0.0))

    a_h = np.ascontiguousarray(
        (A * ASCALE).reshape(NQ, P, DD).transpose(1, 0, 2)
    ).astype(F8)                           # [128, 4, 512]

    in_maps = []
    for b in range(NB):
        Rb = R[b].astype(np.float64)
        rt_h = np.ascontiguousarray(
            Rb.T.reshape(NQ, P, NN).transpose(1, 0, 2)
        ).astype(F8)                       # [128, 4, 2048]
        u = Rb @ (Wv.T @ c) + float(bv @ c)            # [2048]
        m = {"rt": rt_h, "am": a_h}
        if general:
            phi = np.exp((Rb @ (Wk.T @ bq)) * SCALE)   # per-key weight
            m["ub"] = (phi * u).astype(BF).reshape(1, NN)
            m["wb"] = phi.astype(BF).reshape(1, NN)
        else:
            m["ub"] = u.astype(BF).reshape(1, NN)
        in_maps.append(m)
    return general, in_maps, const


def kernel(R, Wq, bq, Wk, bk, Wv, bv, W1, b1, W2, b2):
    R = np.asarray(R, np.float32)
    args = [np.asarray(x, np.float64) for x in (Wq, bq, Wk, bk, Wv, bv, W1, b1, W2, b2)]
    general, in_maps, const = _host_prep(R, *args)

    nc = _get_nc(general)
    res = run_bass_kernel_spmd(nc, in_maps, core_ids=list(range(N_CORES)))
    outs = np.stack([res.results[b]["out"] for b in range(NB)])  # [8,2,128,16]
    s = outs[:, 0].transpose(0, 2, 1).reshape(NB, NN)
    r = outs[:, 1].transpose(0, 2, 1).reshape(NB, NN)
    return (s / r + np.float32(const)).astype(np.float32)
